# revision 10
# baseline (speedup 1.0000x reference)
"""Trainium2 Bass kernel for nn_Attention_66563403153646.

Dense transformer attention block with rotary embeddings + gated adapter
(prefix) attention, fp32 reference:

    y = softmax(rope(x@wq) @ rope(x@wk).T * k + mask) @ (x@wv)
      + gate * softmax(rope(x@wq) @ (adapter@wk).T * k) @ (adapter@wv)
    out = y @ wo

Sharding: 4-way tensor-parallel over heads x 2-way data-parallel over batch
(8 NeuronCores). Each core computes a [S, D] partial of its batch's output
(its 8 heads' contribution through wo); the host sums the 4 TP partials.

Matmuls run as float32r (full fp32 data, PE reduced-precision fast mode;
1 cycle/row for moving dim >= 256 vs 4 for plain fp32). Layouts:
  - x is fed transposed ([D, S]) so projections contract D on partitions.
  - q/k are computed per-head in [HD, S] layout with rope-pair-permuted
    head dims (host permutes wq/wk columns: even rope dims first, odd
    second) so RoPE is two partition-halves of elementwise ops.
  - scores are computed per 128-row q block over 512-col k tiles
    (causal: only tiles <= diagonal; diagonal tile gets the mask band).
  - p blocks are PE-transposed so p@v contracts k on partitions,
    producing out in [HD, S] layout, which is exactly the lhsT layout
    the final wo matmul needs.
"""

import sys

sys.path.insert(0, "/opt/trn_rl_repo")

import math
from dataclasses import dataclass

import numpy as np

import concourse.bass as bass
import concourse.mybir as mybir
import concourse.tile as tile
from concourse import bacc
from concourse.masks import make_identity

f32 = mybir.dt.float32
f32r = mybir.dt.float32r

P = 128


@dataclass(frozen=True)
class Cfg:
    S: int = 2048  # sequence length
    D: int = 4096  # model dim
    HPC: int = 8  # heads per core
    HD: int = 128  # head dim
    AL: int = 10  # adapter len
    mm_dt: object = f32r  # matmul operand dtype

    @property
    def DC(self):  # D chunks of 128 (contraction)
        return self.D // P

    @property
    def DH(self):  # head-slice width
        return self.HPC * self.HD

    @property
    def NQ(self):  # 128-row q blocks
        return self.S // P

    @property
    def NT(self):  # 512-col tiles
        return self.S // 512


def build_nc(cfg: Cfg):
    nc = bacc.Bacc(None, target_bir_lowering=False, debug=False)
    S, D, HPC, HD, AL = cfg.S, cfg.D, cfg.HPC, cfg.HD, cfg.AL
    DC, DH, NQ, NT = cfg.DC, cfg.DH, cfg.NQ, cfg.NT
    mdt = cfg.mm_dt
    inv_sqrt = 1.0 / math.sqrt(HD)

    # ---- I/O ----
    xT_d = nc.dram_tensor("xT", [D, S], mdt, kind="ExternalInput")
    wq_d = nc.dram_tensor("wq", [D, DH], mdt, kind="ExternalInput")
    wk_d = nc.dram_tensor("wk", [D, DH], mdt, kind="ExternalInput")
    wv_d = nc.dram_tensor("wv", [D, DH], mdt, kind="ExternalInput")
    wo_d = nc.dram_tensor("wo", [DH, D], mdt, kind="ExternalInput")
    adT_d = nc.dram_tensor("adT", [D, AL], mdt, kind="ExternalInput")
    cosT_d = nc.dram_tensor("cosT", [HD // 2, S], f32, kind="ExternalInput")
    sinT_d = nc.dram_tensor("sinT", [HD // 2, S], f32, kind="ExternalInput")
    # mask band: for q block i, columns [(i//4)*512, (i//4)*512+512)
    mb_d = nc.dram_tensor("maskband", [NQ, P, 512], f32, kind="ExternalInput")
    gate_d = nc.dram_tensor("gateb", [HPC, P, 1], f32, kind="ExternalInput")
    y_d = nc.dram_tensor("y", [S, D], f32, kind="ExternalOutput")

    ExpF = mybir.ActivationFunctionType.Exp
    AX = mybir.AxisListType.X
    Mul = mybir.AluOpType.mult
    Add = mybir.AluOpType.add

    with tile.TileContext(nc) as tc:
        with (
            tc.tile_pool(name="persist", bufs=1) as persist,
            tc.tile_pool(name="dram", bufs=1, space="DRAM") as dram,
        ):
            # persistent small tiles (cos on partitions 0:64, sin on 64:128)
            cs_sb = persist.tile([P, S], f32)
            HH = HD // 2
            nc.sync.dma_start(cs_sb[0:HH, :], cosT_d[:])
            nc.sync.dma_start(cs_sb[HH:, :], sinT_d[:])
            g_all = persist.tile([P, HPC], f32)
            for h in range(HPC):
                nc.sync.dma_start(g_all[:, h : h + 1], gate_d[h])
            adT_sb = persist.tile([P, DC, AL], mdt)
            nc.sync.dma_start(
                adT_sb[:], adT_d[:].rearrange("(c p) a -> p c a", p=P)
            )
            ident = persist.tile([P, P], f32)
            make_identity(nc, ident)

            # DRAM scratch
            qT_dr = dram.tile([HPC, P, S], mdt)
            kT_dr = dram.tile([HPC, P, S], mdt)
            vT_dr = dram.tile([HPC, P, S], f32)
            oT_dr = dram.tile([HPC, P, S], mdt)
            akT_dr = dram.tile([HPC, P, AL], mdt)
            av_dr = dram.tile([HPC, AL, P], mdt)

            # ================= Phase 1: projections + rope =================
            XS = min(1024, S)  # x s-tile width
            NXS = S // XS
            with (
                tc.tile_pool(name="p1x", bufs=1) as p1x,
                tc.tile_pool(name="p1w", bufs=2) as p1w,
                tc.tile_pool(name="p1o", bufs=2) as p1o,
                tc.tile_pool(name="p1t", bufs=1) as p1t,
                tc.tile_pool(name="p1ps", bufs=4, space="PSUM") as p1ps,
                tc.tile_pool(name="p1psa", bufs=1, space="PSUM") as p1psa,
            ):
                for st in range(NXS):
                    xt = p1x.tile([P, DC, XS], mdt, tag="xt")
                    nc.sync.dma_start(
                        xt[:],
                        xT_d[:, st * XS : (st + 1) * XS].rearrange(
                            "(c p) s -> p c s", p=P
                        ),
                    )
                    for h in range(HPC):
                        for proj, w_dram, out_dr in (
                            ("q", wq_d, qT_dr),
                            ("k", wk_d, kT_dr),
                            ("v", wv_d, vT_dr),
                        ):
                            wt = p1w.tile([P, DC, P], mdt, tag="wt")
                            nc.sync.dma_start(
                                wt[:],
                                w_dram[:, h * HD : (h + 1) * HD].rearrange(
                                    "(c p) m -> p c m", p=P
                                ),
                            )
                            # adapter projections (once, while w resident)
                            if st == 0 and proj == "k":
                                ps_ak = p1psa.tile([P, AL], f32, tag="ps_ak")
                                for c in range(DC):
                                    nc.tensor.matmul(
                                        ps_ak[:],
                                        wt[:, c, :],
                                        adT_sb[:, c, :],
                                        start=(c == 0),
                                        stop=(c == DC - 1),
                                    )
                                akT_sb = p1o.tile([P, AL], mdt, tag="akT")
                                nc.vector.tensor_copy(akT_sb[:], ps_ak[:])
                                nc.sync.dma_start(akT_dr[h], akT_sb[:])
                            if st == 0 and proj == "v":
                                ps_av = p1psa.tile([AL, P], f32, tag="ps_av")
                                for c in range(DC):
                                    nc.tensor.matmul(
                                        ps_av[:],
                                        adT_sb[:, c, :],
                                        wt[:, c, :],
                                        start=(c == 0),
                                        stop=(c == DC - 1),
                                    )
                                av_sb = p1o.tile([AL, P], mdt, tag="av")
                                nc.vector.tensor_copy(av_sb[:], ps_av[:])
                                nc.sync.dma_start(av_dr[h], av_sb[:])

                            for sh in range(XS // 512):
                                soff = st * XS + sh * 512
                                psum = p1ps.tile([P, 512], f32, tag="p1psum")
                                for c in range(DC):
                                    nc.tensor.matmul(
                                        psum[:],
                                        wt[:, c, :],
                                        xt[:, c, sh * 512 : (sh + 1) * 512],
                                        start=(c == 0),
                                        stop=(c == DC - 1),
                                    )
                                if proj == "v":
                                    vt_sb = p1o.tile([P, 512], f32, tag="vt")
                                    nc.scalar.copy(vt_sb[:], psum[:])
                                    nc.sync.dma_start(
                                        out_dr[h, :, soff : soff + 512], vt_sb[:]
                                    )
                                else:
                                    # rope: psum partitions 0:64 = even dims
                                    # (x0), 64:128 = odd dims (x1). All four
                                    # products go to base-0 tmp tiles (PSUM
                                    # x SBUF inputs may differ in base
                                    # partition; SBUF x SBUF may not), the
                                    # two combines are then base-aligned.
                                    c_ap = cs_sb[0:HH, soff : soff + 512]
                                    s_ap = cs_sb[HH:, soff : soff + 512]
                                    x0 = psum[0:HH, :]
                                    x1 = psum[HH : 2 * HH, :]
                                    ta = p1t.tile([HH, 512], f32, tag="ta")
                                    tb = p1t.tile([HH, 512], f32, tag="tb")
                                    tc2 = p1t.tile([HH, 512], f32, tag="tc")
                                    td = p1t.tile([HH, 512], f32, tag="td")
                                    qt_sb = p1o.tile([P, 512], mdt, tag="qkt")
                                    nc.vector.tensor_tensor(ta[:], x0, c_ap, op=Mul)
                                    nc.vector.tensor_tensor(tb[:], x1, s_ap, op=Mul)
                                    nc.vector.tensor_sub(qt_sb[0:HH, :], ta[:], tb[:])
                                    nc.vector.tensor_tensor(tc2[:], x0, s_ap, op=Mul)
                                    nc.vector.tensor_tensor(td[:], x1, c_ap, op=Mul)
                                    nc.vector.tensor_add(qt_sb[HH:, :], tc2[:], td[:])
                                    nc.sync.dma_start(
                                        out_dr[h, :, soff : soff + 512], qt_sb[:]
                                    )

            # ================= Phase 2: attention per head =================
            with (
                tc.tile_pool(name="p2qkv", bufs=2) as p2qkv,
                tc.tile_pool(name="p2vn", bufs=2) as p2vn,
                tc.tile_pool(name="p2sc", bufs=2) as p2sc,
                tc.tile_pool(name="p2pt", bufs=3) as p2pt,
                tc.tile_pool(name="p2sm", bufs=4) as p2sm,
                tc.tile_pool(name="p2o", bufs=2) as p2o,
                tc.tile_pool(name="p2mask", bufs=1) as p2mask,
                tc.tile_pool(name="p2ps_s", bufs=2, space="PSUM") as p2ps_s,
                tc.tile_pool(name="p2ps_t", bufs=2, space="PSUM") as p2ps_t,
                tc.tile_pool(name="p2ps_o", bufs=2, space="PSUM") as p2ps_o,
                tc.tile_pool(name="p2ps_a", bufs=1, space="PSUM") as p2ps_a,
            ):
                mask_sb = p2mask.tile([P, NQ, 512], f32)
                nc.sync.dma_start(
                    mask_sb[:], mb_d[:].rearrange("i p m -> p i m")
                )

                def emit_pv(ph, pQ, sc, apT, v_nat, av):
                    """transpose p blocks + p@v + adapter + evict for (ph,pQ)."""
                    nkb = (pQ + 1) * 4
                    ps_o = p2ps_o.tile([P, 512], f32, tag="ps_o")
                    for jb in range(nkb):
                        pT = p2pt.tile([P, 512], mdt, tag="pT")
                        for qb in range(4):
                            ps_t2 = p2ps_t.tile([P, P], f32, tag="ps_t")
                            nc.tensor.transpose(
                                ps_t2[:],
                                sc[:, qb, jb * P : (jb + 1) * P],
                                ident[:],
                            )
                            nc.any.tensor_copy(
                                pT[:, qb * P : (qb + 1) * P], ps_t2[:]
                            )
                        nc.tensor.matmul(
                            ps_o[:],
                            v_nat[:, jb, :],
                            pT[:],
                            start=(jb == 0),
                            stop=False,
                        )
                    nc.tensor.matmul(ps_o[:], av[:], apT[:], start=False, stop=True)
                    oT_sb = p2o.tile([P, 512], mdt, tag="oT")
                    nc.scalar.copy(oT_sb[:], ps_o[:])
                    nc.sync.dma_start(
                        oT_dr[ph, :, pQ * 512 : (pQ + 1) * 512], oT_sb[:]
                    )

                pending = None
                for h in range(HPC):
                    qT = p2qkv.tile([P, S], mdt, tag="qT")
                    kT = p2qkv.tile([P, S], mdt, tag="kT")
                    vT = p2qkv.tile([P, S], f32, tag="vT")
                    nc.sync.dma_start(qT[:], qT_dr[h])
                    nc.sync.dma_start(kT[:], kT_dr[h])
                    nc.sync.dma_start(vT[:], vT_dr[h])
                    akT = p2sm.tile([P, AL], mdt, tag="akT2")
                    av = p2sm.tile([AL, P], mdt, tag="av2")
                    nc.sync.dma_start(akT[:], akT_dr[h])
                    nc.sync.dma_start(av[:], av_dr[h])

                    # v natural layout [s-block, 16, d]
                    v_nat = p2vn.tile([P, NQ, P], mdt, tag="v_nat")
                    for sb_i in range(NQ):
                        ps_t = p2ps_t.tile([P, P], f32, tag="ps_t")
                        nc.tensor.transpose(
                            ps_t[:], vT[:, sb_i * P : (sb_i + 1) * P], ident[:]
                        )
                        nc.any.tensor_copy(v_nat[:, sb_i, :], ps_t[:])

                    for Q in range(NT):
                        KQ = (Q + 1) * 512
                        nkb = KQ // P
                        # scores / p buffer for this q-tile of 512
                        sc = p2sc.tile([P, 4, S], f32, tag="sc")
                        ssum = p2sm.tile([P, 4], f32, tag="ssum")
                        rsum = p2sm.tile([P, 4], f32, tag="rsum")
                        apT = p2sm.tile([AL, 512], mdt, tag="apT")
                        for qb in range(4):
                            i = Q * 4 + qb
                            qblk = qT[:, i * P : (i + 1) * P]
                            for kt in range(Q + 1):
                                ps_s = p2ps_s.tile([P, 512], f32, tag="ps_s")
                                nc.tensor.matmul(
                                    ps_s[:],
                                    qblk,
                                    kT[:, kt * 512 : (kt + 1) * 512],
                                    start=True,
                                    stop=True,
                                )
                                dst = sc[:, qb, kt * 512 : (kt + 1) * 512]
                                if kt == Q:
                                    # diagonal tile: scale + mask add
                                    nc.vector.scalar_tensor_tensor(
                                        dst,
                                        ps_s[:],
                                        inv_sqrt,
                                        mask_sb[:, i, :],
                                        op0=Mul,
                                        op1=Add,
                                    )
                                else:
                                    nc.scalar.mul(dst, ps_s[:], inv_sqrt)
                            # softmax over [P, KQ]
                            row = sc[:, qb, :KQ]
                            negmax = p2sm.tile([P, 1], f32, tag="negmax")
                            nc.vector.reduce_max(
                                out=negmax[:], in_=row, axis=AX, negate=True
                            )
                            nc.scalar.activation(
                                row,
                                row,
                                ExpF,
                                bias=negmax[:],
                                scale=1.0,
                                accum_out=ssum[:, qb : qb + 1],
                            )
                            nc.vector.reciprocal(
                                rsum[:, qb : qb + 1], ssum[:, qb : qb + 1]
                            )
                            nc.any.tensor_scalar_mul(row, row, rsum[:, qb : qb + 1])

                            # adapter scores for this q block
                            ps_a = p2ps_a.tile([P, AL], f32, tag="ps_a")
                            nc.tensor.matmul(
                                ps_a[:], qblk, akT[:], start=True, stop=True
                            )
                            asm = p2sm.tile([P, AL], f32, tag="asm")
                            nc.scalar.mul(asm[:], ps_a[:], inv_sqrt)
                            anegmax = p2sm.tile([P, 1], f32, tag="anegmax")
                            nc.vector.reduce_max(
                                out=anegmax[:], in_=asm[:], axis=AX, negate=True
                            )
                            asum = p2sm.tile([P, 1], f32, tag="asum")
                            nc.scalar.activation(
                                asm[:],
                                asm[:],
                                ExpF,
                                bias=anegmax[:],
                                scale=1.0,
                                accum_out=asum[:],
                            )
                            arec = p2sm.tile([P, 1], f32, tag="arec")
                            nc.vector.reciprocal(arec[:], asum[:])
                            nc.vector.tensor_tensor(
                                arec[:], arec[:], g_all[:, h : h + 1], op=Mul
                            )
                            nc.any.tensor_scalar_mul(asm[:], asm[:], arec[:])
                            # transpose adapter probs -> [AL, 128]
                            ps_apt = p2ps_a.tile([P, P], f32, tag="ps_apt")
                            nc.tensor.transpose(ps_apt[:AL, :], asm[:], ident[:])
                            nc.any.tensor_copy(
                                apT[:, qb * P : (qb + 1) * P], ps_apt[:AL, :]
                            )

                        # pipeline: transpose+pv of the PREVIOUS q-tile now,
                        # so PE fills the softmax wait with useful work
                        if pending is not None:
                            emit_pv(*pending)
                        pending = (h, Q, sc, apT, v_nat, av)
                if pending is not None:
                    emit_pv(*pending)

            # ================= Phase 3: out @ wo =================
            with (
                tc.tile_pool(name="p3o", bufs=1) as p3o,
                tc.tile_pool(name="p3w", bufs=2) as p3w,
                tc.tile_pool(name="p3y", bufs=3) as p3y,
                tc.tile_pool(name="p3ps", bufs=4, space="PSUM") as p3ps,
            ):
                oTs = []
                for h in range(HPC):
                    ot = p3o.tile([P, S], mdt, name=f"p3ot{h}")
                    nc.sync.dma_start(ot[:], oT_dr[h])
                    oTs.append(ot)
                WC = DH // P  # wo row chunks == HPC
                for et in range(D // 512):
                    wo_t = p3w.tile([P, WC, 512], mdt, tag="wo_t")
                    nc.sync.dma_start(
                        wo_t[:],
                        wo_d[:, et * 512 : (et + 1) * 512].rearrange(
                            "(c p) e -> p c e", p=P
                        ),
                    )
                    for st in range(NQ):
                        ps_y = p3ps.tile([P, 512], f32, tag="ps_y")
                        for h in range(HPC):
                            nc.tensor.matmul(
                                ps_y[:],
                                oTs[h][:, st * P : (st + 1) * P],
                                wo_t[:, h, :],
                                start=(h == 0),
                                stop=(h == HPC - 1),
                            )
                        y_sb = p3y.tile([P, 512], f32, tag="y_sb")
                        nc.scalar.copy(y_sb[:], ps_y[:])
                        nc.sync.dma_start(
                            y_d[st * P : (st + 1) * P, et * 512 : (et + 1) * 512],
                            y_sb[:],
                        )

    nc.compile()
    return nc


# ====================== host side: sharding + runner ======================

B, S, D, H = 2, 2048, 4096, 32
HD = D // H
AL = 10
N_CORES = 8
TP = 4  # head groups
HPC = H // TP  # 8 heads per core

_RUNNER = None


def _make_runner(nc, n_cores=N_CORES):
    import jax
    from jax.sharding import Mesh, PartitionSpec
    from jax.experimental.shard_map import shard_map

    from concourse import bass2jax
    from concourse.bass2jax import _bass_exec_p, install_neuronx_cc_hook

    install_neuronx_cc_hook()
    partition_name = nc.partition_id_tensor.name if nc.partition_id_tensor else None

    in_names, out_names, out_avals = [], [], []
    for alloc in nc.m.functions[0].allocations:
        if not isinstance(alloc, mybir.MemoryLocationSet):
            continue
        name = alloc.memorylocations[0].name
        if alloc.kind == "ExternalInput":
            if name != partition_name:
                in_names.append(name)
        elif alloc.kind == "ExternalOutput":
            out_names.append(name)
            out_avals.append(
                jax.core.ShapedArray(
                    tuple(alloc.tensor_shape), mybir.dt.np(alloc.dtype)
                )
            )
    n_params = len(in_names)
    n_outs = len(out_avals)
    all_in_names = list(in_names) + list(out_names)
    if partition_name is not None:
        all_in_names.append(partition_name)

    def _body(*args):
        operands = list(args)
        if partition_name is not None:
            operands.append(bass2jax.partition_id_tensor())
        outs = _bass_exec_p.bind(
            *operands,
            out_avals=tuple(out_avals),
            in_names=tuple(all_in_names),
            out_names=tuple(out_names),
            lowering_input_output_aliases=(),
            sim_require_finite=True,
            sim_require_nnan=True,
            nc=nc,
        )
        return tuple(outs)

    devices = jax.devices()[:n_cores]
    mesh = Mesh(np.asarray(devices), ("core",))
    fn = jax.jit(
        shard_map(
            _body,
            mesh=mesh,
            in_specs=(PartitionSpec("core"),) * (n_params + n_outs),
            out_specs=(PartitionSpec("core"),) * n_outs,
            check_rep=False,
        ),
        keep_unused=True,
    )

    class Runner:
        in_names_ = in_names
        out_names_ = out_names

        def prep(self, in_maps):
            import jax as _jax

            concat_in = [
                np.concatenate(
                    [np.ascontiguousarray(in_maps[c][n]) for c in range(n_cores)],
                    axis=0,
                )
                for n in in_names
            ]
            concat_zero = [
                np.zeros((n_cores * a.shape[0], *a.shape[1:]), a.dtype)
                for a in out_avals
            ]
            shardings = [
                _jax.sharding.NamedSharding(mesh, PartitionSpec("core"))
            ] * (n_params + n_outs)
            return _jax.device_put(concat_in + concat_zero, shardings)

        def run(self, args):
            import jax as _jax

            outs = fn(*args)
            _jax.block_until_ready(outs)
            return [
                {
                    n: np.asarray(outs[i]).reshape(n_cores, *out_avals[i].shape)[c]
                    for i, n in enumerate(out_names)
                }
                for c in range(n_cores)
            ]

        def time_pipelined(self, args, reps=10, warmup=1):
            import time as _time

            import jax as _jax

            for _ in range(warmup):
                _jax.block_until_ready(fn(*args))
            t0 = _time.perf_counter()
            outs = None
            for _ in range(reps):
                outs = fn(*args)
            _jax.block_until_ready(outs)
            return (_time.perf_counter() - t0) / reps

    return Runner()


def _shard_inputs(x, cos, sin, mask, wq, wk, wv, wo, gate, adapter):
    """Build the 8 per-core input maps."""
    # rope permutation of head-dim columns: even dims first, odd second
    perm = np.concatenate(
        [np.arange(0, HD, 2), np.arange(1, HD, 2)]
    )  # within one head
    col_perm = np.concatenate(
        [h * HD + perm for h in range(H)]
    )  # all heads, head-major
    wq_p = np.ascontiguousarray(wq[:, col_perm], dtype=np.float32)
    wk_p = np.ascontiguousarray(wk[:, col_perm], dtype=np.float32)

    cosT = np.ascontiguousarray(cos.T, dtype=np.float32)  # [64, S]
    sinT = np.ascontiguousarray(sin.T, dtype=np.float32)
    adT = np.ascontiguousarray(adapter[0].T, dtype=np.float32)  # [D, AL]

    m = np.asarray(mask, dtype=np.float32)[0, 0]  # [S, S]
    NQ = S // P
    maskband = np.empty((NQ, P, 512), dtype=np.float32)
    for i in range(NQ):
        c0 = (i // 4) * 512
        maskband[i] = m[i * P : (i + 1) * P, c0 : c0 + 512]

    gate_v = np.asarray(gate, dtype=np.float32).reshape(H)  # per head

    xT = [
        np.ascontiguousarray(np.asarray(x[b], dtype=np.float32).T) for b in range(B)
    ]

    in_maps = []
    for c in range(N_CORES):
        b = c // TP
        g = c % TP
        hs = g * HPC * HD  # column slice start
        gateb = np.ascontiguousarray(
            np.repeat(gate_v[g * HPC : (g + 1) * HPC, None], P, axis=1)[..., None]
        ).astype(np.float32)  # [HPC, P, 1]
        in_maps.append(
            {
                "xT": xT[b],
                "wq": np.ascontiguousarray(wq_p[:, hs : hs + HPC * HD]),
                "wk": np.ascontiguousarray(wk_p[:, hs : hs + HPC * HD]),
                "wv": np.ascontiguousarray(
                    np.asarray(wv, dtype=np.float32)[:, hs : hs + HPC * HD]
                ),
                "wo": np.ascontiguousarray(
                    np.asarray(wo, dtype=np.float32)[hs : hs + HPC * HD, :]
                ),
                "adT": adT,
                "cosT": cosT,
                "sinT": sinT,
                "maskband": maskband,
                "gateb": gateb,
            }
        )
    return in_maps


def get_runner():
    global _RUNNER
    if _RUNNER is None:
        nc = build_nc(Cfg())
        _RUNNER = _make_runner(nc)
    return _RUNNER


def kernel(**inputs) -> np.ndarray:
    x = np.asarray(inputs["x"])
    in_maps = _shard_inputs(
        x,
        inputs["cos"],
        inputs["sin"],
        inputs["mask"],
        inputs["wq"],
        inputs["wk"],
        inputs["wv"],
        inputs["wo"],
        inputs["gate"],
        inputs["adapter"],
    )
    runner = get_runner()
    args = runner.prep(in_maps)
    outs = runner.run(args)
    y = np.zeros((B, S, D), dtype=np.float32)
    for c in range(N_CORES):
        y[c // TP] += outs[c]["y"]
    return y


# revision 11
# speedup vs baseline: 1.5198x; 1.5198x over previous
"""Trainium2 Bass kernel for nn_Attention_66563403153646.

Dense transformer attention block with rotary embeddings + gated adapter
(prefix) attention, fp32 reference:

    y = softmax(rope(x@wq) @ rope(x@wk).T * k + mask) @ (x@wv)
      + gate * softmax(rope(x@wq) @ (adapter@wk).T * k) @ (adapter@wv)
    out = y @ wo

Sharding: 4-way tensor-parallel over heads x 2-way data-parallel over batch
(8 NeuronCores). Each core computes a [S, D] partial of its batch's output
(its 8 heads' contribution through wo); the host sums the 4 TP partials.

Matmuls run as float32r (full fp32 data, PE reduced-precision fast mode;
1 cycle/row for moving dim >= 256 vs 4 for plain fp32). Layouts:
  - x is fed transposed ([D, S]) so projections contract D on partitions.
  - q/k are computed per-head in [HD, S] layout with rope-pair-permuted
    head dims (host permutes wq/wk columns: even rope dims first, odd
    second) so RoPE is two partition-halves of elementwise ops.
  - scores are computed per 128-row q block over 512-col k tiles
    (causal: only tiles <= diagonal; diagonal tile gets the mask band).
  - p blocks are PE-transposed so p@v contracts k on partitions,
    producing out in [HD, S] layout, which is exactly the lhsT layout
    the final wo matmul needs.
"""

import sys

sys.path.insert(0, "/opt/trn_rl_repo")

import math
from dataclasses import dataclass

import numpy as np

import concourse.bass as bass
import concourse.mybir as mybir
import concourse.tile as tile
from concourse import bacc
from concourse.masks import make_identity

f32 = mybir.dt.float32
f32r = mybir.dt.float32r

P = 128


@dataclass(frozen=True)
class Cfg:
    S: int = 2048  # sequence length
    D: int = 4096  # model dim
    HPC: int = 8  # heads per core
    HD: int = 128  # head dim
    AL: int = 10  # adapter len
    mm_dt: object = f32r  # matmul operand dtype

    @property
    def DC(self):  # D chunks of 128 (contraction)
        return self.D // P

    @property
    def DH(self):  # head-slice width
        return self.HPC * self.HD

    @property
    def NQ(self):  # 128-row q blocks
        return self.S // P

    @property
    def NT(self):  # 512-col tiles
        return self.S // 512


def build_nc(cfg: Cfg, phases=(1, 2, 3)):
    nc = bacc.Bacc(None, target_bir_lowering=False, debug=False)
    S, D, HPC, HD, AL = cfg.S, cfg.D, cfg.HPC, cfg.HD, cfg.AL
    DC, DH, NQ, NT = cfg.DC, cfg.DH, cfg.NQ, cfg.NT
    mdt = cfg.mm_dt
    inv_sqrt = 1.0 / math.sqrt(HD)

    # ---- I/O ----
    xT_d = nc.dram_tensor("xT", [D, S], mdt, kind="ExternalInput")
    wq_d = nc.dram_tensor("wq", [D, DH], mdt, kind="ExternalInput")
    wk_d = nc.dram_tensor("wk", [D, DH], mdt, kind="ExternalInput")
    wv_d = nc.dram_tensor("wv", [D, DH], mdt, kind="ExternalInput")
    wo_d = nc.dram_tensor("wo", [DH, D], mdt, kind="ExternalInput")
    adT_d = nc.dram_tensor("adT", [D, AL], mdt, kind="ExternalInput")
    cosT_d = nc.dram_tensor("cosT", [HD // 2, S], f32, kind="ExternalInput")
    sinT_d = nc.dram_tensor("sinT", [HD // 2, S], f32, kind="ExternalInput")
    # mask band: for q block i, columns [(i//4)*512, (i//4)*512+512)
    mb_d = nc.dram_tensor("maskband", [NQ, P, 512], f32, kind="ExternalInput")
    gate_d = nc.dram_tensor("gateb", [HPC, P, 1], f32, kind="ExternalInput")
    y_d = nc.dram_tensor("y", [S, D], f32, kind="ExternalOutput")

    ExpF = mybir.ActivationFunctionType.Exp
    AX = mybir.AxisListType.X
    Mul = mybir.AluOpType.mult
    Add = mybir.AluOpType.add

    with tile.TileContext(nc) as tc:
        with (
            tc.tile_pool(name="persist", bufs=1) as persist,
            tc.tile_pool(name="dram", bufs=1, space="DRAM") as dram,
        ):
            # persistent small tiles (cos on partitions 0:64, sin on 64:128)
            cs_sb = persist.tile([P, S], f32)
            HH = HD // 2
            nc.sync.dma_start(cs_sb[0:HH, :], cosT_d[:])
            nc.sync.dma_start(cs_sb[HH:, :], sinT_d[:])
            g_all = persist.tile([P, HPC], f32)
            for h in range(HPC):
                nc.sync.dma_start(g_all[:, h : h + 1], gate_d[h])
            adT_sb = persist.tile([P, DC, AL], mdt)
            nc.sync.dma_start(
                adT_sb[:], adT_d[:].rearrange("(c p) a -> p c a", p=P)
            )
            ident = persist.tile([P, P], f32)
            make_identity(nc, ident)

            # DRAM scratch
            qT_dr = dram.tile([HPC, P, S], mdt)
            kT_dr = dram.tile([HPC, P, S], mdt)
            vT_dr = dram.tile([HPC, P, S], f32)
            oT_dr = dram.tile([HPC, P, S], mdt)
            akT_dr = dram.tile([HPC, P, AL], mdt)
            av_dr = dram.tile([HPC, AL, P], mdt)

            # ================= Phase 1: projections + rope =================
            XS = min(1024, S)  # x s-tile width
            NXS = S // XS if 1 in phases else 0
            with (
                tc.tile_pool(name="p1x", bufs=1) as p1x,
                tc.tile_pool(name="p1w", bufs=2) as p1w,
                tc.tile_pool(name="p1o", bufs=2) as p1o,
                tc.tile_pool(name="p1t", bufs=1) as p1t,
                tc.tile_pool(name="p1ps", bufs=4, space="PSUM") as p1ps,
                tc.tile_pool(name="p1psa", bufs=1, space="PSUM") as p1psa,
            ):
                for st in range(NXS):
                    xt = p1x.tile([P, DC, XS], mdt, tag="xt")
                    nc.sync.dma_start(
                        xt[:],
                        xT_d[:, st * XS : (st + 1) * XS].rearrange(
                            "(c p) s -> p c s", p=P
                        ),
                    )
                    for h in range(HPC):
                        for proj, w_dram, out_dr in (
                            ("q", wq_d, qT_dr),
                            ("k", wk_d, kT_dr),
                            ("v", wv_d, vT_dr),
                        ):
                            wt = p1w.tile([P, DC, P], mdt, tag="wt")
                            nc.sync.dma_start(
                                wt[:],
                                w_dram[:, h * HD : (h + 1) * HD].rearrange(
                                    "(c p) m -> p c m", p=P
                                ),
                            )
                            # adapter projections (once, while w resident)
                            if st == 0 and proj == "k":
                                ps_ak = p1psa.tile([P, AL], f32, tag="ps_ak")
                                for c in range(DC):
                                    nc.tensor.matmul(
                                        ps_ak[:],
                                        wt[:, c, :],
                                        adT_sb[:, c, :],
                                        start=(c == 0),
                                        stop=(c == DC - 1),
                                    )
                                akT_sb = p1o.tile([P, AL], mdt, tag="akT")
                                nc.vector.tensor_copy(akT_sb[:], ps_ak[:])
                                nc.sync.dma_start(akT_dr[h], akT_sb[:])
                            if st == 0 and proj == "v":
                                ps_av = p1psa.tile([AL, P], f32, tag="ps_av")
                                for c in range(DC):
                                    nc.tensor.matmul(
                                        ps_av[:],
                                        adT_sb[:, c, :],
                                        wt[:, c, :],
                                        start=(c == 0),
                                        stop=(c == DC - 1),
                                    )
                                av_sb = p1o.tile([AL, P], mdt, tag="av")
                                nc.vector.tensor_copy(av_sb[:], ps_av[:])
                                nc.sync.dma_start(av_dr[h], av_sb[:])

                            for sh in range(XS // 512):
                                soff = st * XS + sh * 512
                                psum = p1ps.tile([P, 512], f32, tag="p1psum")
                                for c in range(DC):
                                    nc.tensor.matmul(
                                        psum[:],
                                        wt[:, c, :],
                                        xt[:, c, sh * 512 : (sh + 1) * 512],
                                        start=(c == 0),
                                        stop=(c == DC - 1),
                                    )
                                if proj == "v":
                                    vt_sb = p1o.tile([P, 512], f32, tag="vt")
                                    nc.scalar.copy(vt_sb[:], psum[:])
                                    nc.sync.dma_start(
                                        out_dr[h, :, soff : soff + 512], vt_sb[:]
                                    )
                                else:
                                    # rope: psum partitions 0:64 = even dims
                                    # (x0), 64:128 = odd dims (x1). All four
                                    # products go to base-0 tmp tiles (PSUM
                                    # x SBUF inputs may differ in base
                                    # partition; SBUF x SBUF may not), the
                                    # two combines are then base-aligned.
                                    c_ap = cs_sb[0:HH, soff : soff + 512]
                                    s_ap = cs_sb[HH:, soff : soff + 512]
                                    x0 = psum[0:HH, :]
                                    x1 = psum[HH : 2 * HH, :]
                                    ta = p1t.tile([HH, 512], f32, tag="ta")
                                    tb = p1t.tile([HH, 512], f32, tag="tb")
                                    tc2 = p1t.tile([HH, 512], f32, tag="tc")
                                    td = p1t.tile([HH, 512], f32, tag="td")
                                    qt_sb = p1o.tile([P, 512], mdt, tag="qkt")
                                    nc.vector.tensor_tensor(ta[:], x0, c_ap, op=Mul)
                                    nc.vector.tensor_tensor(tb[:], x1, s_ap, op=Mul)
                                    nc.vector.tensor_sub(qt_sb[0:HH, :], ta[:], tb[:])
                                    nc.vector.tensor_tensor(tc2[:], x0, s_ap, op=Mul)
                                    nc.vector.tensor_tensor(td[:], x1, c_ap, op=Mul)
                                    nc.vector.tensor_add(qt_sb[HH:, :], tc2[:], td[:])
                                    nc.sync.dma_start(
                                        out_dr[h, :, soff : soff + 512], qt_sb[:]
                                    )

            # ================= Phase 2: attention per head =================
            HPC2 = HPC if 2 in phases else 0
            with (
                tc.tile_pool(name="p2qkv", bufs=2) as p2qkv,
                tc.tile_pool(name="p2vn", bufs=2) as p2vn,
                tc.tile_pool(name="p2sc", bufs=2) as p2sc,
                tc.tile_pool(name="p2pt", bufs=3) as p2pt,
                tc.tile_pool(name="p2sm", bufs=4) as p2sm,
                tc.tile_pool(name="p2o", bufs=2) as p2o,
                tc.tile_pool(name="p2mask", bufs=1) as p2mask,
                tc.tile_pool(name="p2ps_s", bufs=2, space="PSUM") as p2ps_s,
                tc.tile_pool(name="p2ps_t", bufs=2, space="PSUM") as p2ps_t,
                tc.tile_pool(name="p2ps_o", bufs=2, space="PSUM") as p2ps_o,
                tc.tile_pool(name="p2ps_a", bufs=1, space="PSUM") as p2ps_a,
            ):
                mask_sb = p2mask.tile([P, NQ, 512], f32)
                nc.sync.dma_start(
                    mask_sb[:], mb_d[:].rearrange("i p m -> p i m")
                )

                def emit_pv(ph, pQ, sc, apT, v_nat, av):
                    """transpose p blocks + p@v + adapter + evict for (ph,pQ)."""
                    nkb = (pQ + 1) * 4
                    ps_o = p2ps_o.tile([P, 512], f32, tag="ps_o")
                    for jb in range(nkb):
                        pT = p2pt.tile([P, 512], mdt, tag="pT")
                        for qb in range(4):
                            ps_t2 = p2ps_t.tile([P, P], f32, tag="ps_t")
                            nc.tensor.transpose(
                                ps_t2[:],
                                sc[:, qb, jb * P : (jb + 1) * P],
                                ident[:],
                            )
                            nc.any.tensor_copy(
                                pT[:, qb * P : (qb + 1) * P], ps_t2[:]
                            )
                        nc.tensor.matmul(
                            ps_o[:],
                            v_nat[:, jb, :],
                            pT[:],
                            start=(jb == 0),
                            stop=False,
                        )
                    nc.tensor.matmul(ps_o[:], av[:], apT[:], start=False, stop=True)
                    oT_sb = p2o.tile([P, 512], mdt, tag="oT")
                    nc.scalar.copy(oT_sb[:], ps_o[:])
                    nc.sync.dma_start(
                        oT_dr[ph, :, pQ * 512 : (pQ + 1) * 512], oT_sb[:]
                    )

                pending = None
                for h in range(HPC2):
                    qT = p2qkv.tile([P, S], mdt, tag="qT")
                    kT = p2qkv.tile([P, S], mdt, tag="kT")
                    vT = p2qkv.tile([P, S], f32, tag="vT")
                    nc.sync.dma_start(qT[:], qT_dr[h])
                    nc.sync.dma_start(kT[:], kT_dr[h])
                    nc.sync.dma_start(vT[:], vT_dr[h])
                    akT = p2sm.tile([P, AL], mdt, tag="akT2")
                    av = p2sm.tile([AL, P], mdt, tag="av2")
                    nc.sync.dma_start(akT[:], akT_dr[h])
                    nc.sync.dma_start(av[:], av_dr[h])

                    # v natural layout [s-block, 16, d]
                    v_nat = p2vn.tile([P, NQ, P], mdt, tag="v_nat")
                    for sb_i in range(NQ):
                        ps_t = p2ps_t.tile([P, P], f32, tag="ps_t")
                        nc.tensor.transpose(
                            ps_t[:], vT[:, sb_i * P : (sb_i + 1) * P], ident[:]
                        )
                        nc.any.tensor_copy(v_nat[:, sb_i, :], ps_t[:])

                    for Q in range(NT):
                        KQ = (Q + 1) * 512
                        nkb = KQ // P
                        # scores / p buffer for this q-tile of 512
                        sc = p2sc.tile([P, 4, S], f32, tag="sc")
                        ssum = p2sm.tile([P, 4], f32, tag="ssum")
                        rsum = p2sm.tile([P, 4], f32, tag="rsum")
                        apT = p2sm.tile([AL, 512], mdt, tag="apT")
                        for qb in range(4):
                            i = Q * 4 + qb
                            qblk = qT[:, i * P : (i + 1) * P]
                            for kt in range(Q + 1):
                                ps_s = p2ps_s.tile([P, 512], f32, tag="ps_s")
                                nc.tensor.matmul(
                                    ps_s[:],
                                    qblk,
                                    kT[:, kt * 512 : (kt + 1) * 512],
                                    start=True,
                                    stop=True,
                                )
                                dst = sc[:, qb, kt * 512 : (kt + 1) * 512]
                                if kt == Q:
                                    # diagonal tile: scale + mask add
                                    nc.vector.scalar_tensor_tensor(
                                        dst,
                                        ps_s[:],
                                        inv_sqrt,
                                        mask_sb[:, i, :],
                                        op0=Mul,
                                        op1=Add,
                                    )
                                else:
                                    nc.scalar.mul(dst, ps_s[:], inv_sqrt)
                            # softmax over [P, KQ]
                            row = sc[:, qb, :KQ]
                            negmax = p2sm.tile([P, 1], f32, tag="negmax")
                            nc.vector.reduce_max(
                                out=negmax[:], in_=row, axis=AX, negate=True
                            )
                            nc.scalar.activation(
                                row,
                                row,
                                ExpF,
                                bias=negmax[:],
                                scale=1.0,
                                accum_out=ssum[:, qb : qb + 1],
                            )
                            nc.vector.reciprocal(
                                rsum[:, qb : qb + 1], ssum[:, qb : qb + 1]
                            )
                            nc.any.tensor_scalar_mul(row, row, rsum[:, qb : qb + 1])

                            # adapter scores for this q block
                            ps_a = p2ps_a.tile([P, AL], f32, tag="ps_a")
                            nc.tensor.matmul(
                                ps_a[:], qblk, akT[:], start=True, stop=True
                            )
                            asm = p2sm.tile([P, AL], f32, tag="asm")
                            nc.scalar.mul(asm[:], ps_a[:], inv_sqrt)
                            anegmax = p2sm.tile([P, 1], f32, tag="anegmax")
                            nc.vector.reduce_max(
                                out=anegmax[:], in_=asm[:], axis=AX, negate=True
                            )
                            asum = p2sm.tile([P, 1], f32, tag="asum")
                            nc.scalar.activation(
                                asm[:],
                                asm[:],
                                ExpF,
                                bias=anegmax[:],
                                scale=1.0,
                                accum_out=asum[:],
                            )
                            arec = p2sm.tile([P, 1], f32, tag="arec")
                            nc.vector.reciprocal(arec[:], asum[:])
                            nc.vector.tensor_tensor(
                                arec[:], arec[:], g_all[:, h : h + 1], op=Mul
                            )
                            nc.any.tensor_scalar_mul(asm[:], asm[:], arec[:])
                            # transpose adapter probs -> [AL, 128]
                            ps_apt = p2ps_a.tile([P, P], f32, tag="ps_apt")
                            nc.tensor.transpose(ps_apt[:AL, :], asm[:], ident[:])
                            nc.any.tensor_copy(
                                apT[:, qb * P : (qb + 1) * P], ps_apt[:AL, :]
                            )

                        # pipeline: transpose+pv of the PREVIOUS q-tile now,
                        # so PE fills the softmax wait with useful work
                        if pending is not None:
                            emit_pv(*pending)
                        pending = (h, Q, sc, apT, v_nat, av)
                if pending is not None:
                    emit_pv(*pending)

            # ================= Phase 3: out @ wo =================
            HPC3 = HPC if 3 in phases else 0
            with (
                tc.tile_pool(name="p3o", bufs=1) as p3o,
                tc.tile_pool(name="p3w", bufs=2) as p3w,
                tc.tile_pool(name="p3y", bufs=3) as p3y,
                tc.tile_pool(name="p3ps", bufs=4, space="PSUM") as p3ps,
            ):
                oTs = []
                for h in range(HPC3):
                    ot = p3o.tile([P, S], mdt, name=f"p3ot{h}")
                    nc.sync.dma_start(ot[:], oT_dr[h])
                    oTs.append(ot)
                WC = DH // P  # wo row chunks == HPC
                for et in range(D // 512 if 3 in phases else 0):
                    wo_t = p3w.tile([P, WC, 512], mdt, tag="wo_t")
                    nc.sync.dma_start(
                        wo_t[:],
                        wo_d[:, et * 512 : (et + 1) * 512].rearrange(
                            "(c p) e -> p c e", p=P
                        ),
                    )
                    for st in range(NQ):
                        ps_y = p3ps.tile([P, 512], f32, tag="ps_y")
                        for h in range(HPC):
                            nc.tensor.matmul(
                                ps_y[:],
                                oTs[h][:, st * P : (st + 1) * P],
                                wo_t[:, h, :],
                                start=(h == 0),
                                stop=(h == HPC - 1),
                            )
                        y_sb = p3y.tile([P, 512], f32, tag="y_sb")
                        nc.scalar.copy(y_sb[:], ps_y[:])
                        nc.sync.dma_start(
                            y_d[st * P : (st + 1) * P, et * 512 : (et + 1) * 512],
                            y_sb[:],
                        )

    nc.compile()
    return nc


# ====================== host side: sharding + runner ======================

B, S, D, H = 2, 2048, 4096, 32
HD = D // H
AL = 10
N_CORES = 8
TP = 4  # head groups
HPC = H // TP  # 8 heads per core

_RUNNER = None


def _make_runner(nc, n_cores=N_CORES):
    import jax
    from jax.sharding import Mesh, PartitionSpec
    from jax.experimental.shard_map import shard_map

    from concourse import bass2jax
    from concourse.bass2jax import _bass_exec_p, install_neuronx_cc_hook

    install_neuronx_cc_hook()
    partition_name = nc.partition_id_tensor.name if nc.partition_id_tensor else None

    in_names, out_names, out_avals = [], [], []
    for alloc in nc.m.functions[0].allocations:
        if not isinstance(alloc, mybir.MemoryLocationSet):
            continue
        name = alloc.memorylocations[0].name
        if alloc.kind == "ExternalInput":
            if name != partition_name:
                in_names.append(name)
        elif alloc.kind == "ExternalOutput":
            out_names.append(name)
            out_avals.append(
                jax.core.ShapedArray(
                    tuple(alloc.tensor_shape), mybir.dt.np(alloc.dtype)
                )
            )
    n_params = len(in_names)
    n_outs = len(out_avals)
    all_in_names = list(in_names) + list(out_names)
    if partition_name is not None:
        all_in_names.append(partition_name)

    def _body(*args):
        operands = list(args)
        if partition_name is not None:
            operands.append(bass2jax.partition_id_tensor())
        outs = _bass_exec_p.bind(
            *operands,
            out_avals=tuple(out_avals),
            in_names=tuple(all_in_names),
            out_names=tuple(out_names),
            lowering_input_output_aliases=(),
            sim_require_finite=True,
            sim_require_nnan=True,
            nc=nc,
        )
        return tuple(outs)

    devices = jax.devices()[:n_cores]
    mesh = Mesh(np.asarray(devices), ("core",))
    fn = jax.jit(
        shard_map(
            _body,
            mesh=mesh,
            in_specs=(PartitionSpec("core"),) * (n_params + n_outs),
            out_specs=(PartitionSpec("core"),) * n_outs,
            check_rep=False,
        ),
        keep_unused=True,
    )

    class Runner:
        in_names_ = in_names
        out_names_ = out_names

        def prep(self, in_maps):
            import jax as _jax

            concat_in = [
                np.concatenate(
                    [np.ascontiguousarray(in_maps[c][n]) for c in range(n_cores)],
                    axis=0,
                )
                for n in in_names
            ]
            concat_zero = [
                np.zeros((n_cores * a.shape[0], *a.shape[1:]), a.dtype)
                for a in out_avals
            ]
            shardings = [
                _jax.sharding.NamedSharding(mesh, PartitionSpec("core"))
            ] * (n_params + n_outs)
            return _jax.device_put(concat_in + concat_zero, shardings)

        def run(self, args):
            import jax as _jax

            outs = fn(*args)
            _jax.block_until_ready(outs)
            return [
                {
                    n: np.asarray(outs[i]).reshape(n_cores, *out_avals[i].shape)[c]
                    for i, n in enumerate(out_names)
                }
                for c in range(n_cores)
            ]

        def time_pipelined(self, args, reps=10, warmup=1):
            import time as _time

            import jax as _jax

            for _ in range(warmup):
                _jax.block_until_ready(fn(*args))
            t0 = _time.perf_counter()
            outs = None
            for _ in range(reps):
                outs = fn(*args)
            _jax.block_until_ready(outs)
            return (_time.perf_counter() - t0) / reps

    return Runner()


def _shard_inputs(x, cos, sin, mask, wq, wk, wv, wo, gate, adapter):
    """Build the 8 per-core input maps."""
    # rope permutation of head-dim columns: even dims first, odd second
    perm = np.concatenate(
        [np.arange(0, HD, 2), np.arange(1, HD, 2)]
    )  # within one head
    col_perm = np.concatenate(
        [h * HD + perm for h in range(H)]
    )  # all heads, head-major
    wq_p = np.ascontiguousarray(wq[:, col_perm], dtype=np.float32)
    wk_p = np.ascontiguousarray(wk[:, col_perm], dtype=np.float32)

    cosT = np.ascontiguousarray(cos.T, dtype=np.float32)  # [64, S]
    sinT = np.ascontiguousarray(sin.T, dtype=np.float32)
    adT = np.ascontiguousarray(adapter[0].T, dtype=np.float32)  # [D, AL]

    m = np.asarray(mask, dtype=np.float32)[0, 0]  # [S, S]
    NQ = S // P
    maskband = np.empty((NQ, P, 512), dtype=np.float32)
    for i in range(NQ):
        c0 = (i // 4) * 512
        maskband[i] = m[i * P : (i + 1) * P, c0 : c0 + 512]

    gate_v = np.asarray(gate, dtype=np.float32).reshape(H)  # per head

    xT = [
        np.ascontiguousarray(np.asarray(x[b], dtype=np.float32).T) for b in range(B)
    ]

    in_maps = []
    for c in range(N_CORES):
        b = c // TP
        g = c % TP
        hs = g * HPC * HD  # column slice start
        gateb = np.ascontiguousarray(
            np.repeat(gate_v[g * HPC : (g + 1) * HPC, None], P, axis=1)[..., None]
        ).astype(np.float32)  # [HPC, P, 1]
        in_maps.append(
            {
                "xT": xT[b],
                "wq": np.ascontiguousarray(wq_p[:, hs : hs + HPC * HD]),
                "wk": np.ascontiguousarray(wk_p[:, hs : hs + HPC * HD]),
                "wv": np.ascontiguousarray(
                    np.asarray(wv, dtype=np.float32)[:, hs : hs + HPC * HD]
                ),
                "wo": np.ascontiguousarray(
                    np.asarray(wo, dtype=np.float32)[hs : hs + HPC * HD, :]
                ),
                "adT": adT,
                "cosT": cosT,
                "sinT": sinT,
                "maskband": maskband,
                "gateb": gateb,
            }
        )
    return in_maps


def get_runner():
    global _RUNNER
    if _RUNNER is None:
        nc = build_nc(Cfg())
        _RUNNER = _make_runner(nc)
    return _RUNNER


def kernel(**inputs) -> np.ndarray:
    x = np.asarray(inputs["x"])
    in_maps = _shard_inputs(
        x,
        inputs["cos"],
        inputs["sin"],
        inputs["mask"],
        inputs["wq"],
        inputs["wk"],
        inputs["wv"],
        inputs["wo"],
        inputs["gate"],
        inputs["adapter"],
    )
    runner = get_runner()
    args = runner.prep(in_maps)
    outs = runner.run(args)
    y = np.zeros((B, S, D), dtype=np.float32)
    for c in range(N_CORES):
        y[c // TP] += outs[c]["y"]
    return y


# revision 13
# speedup vs baseline: 3.9909x; 2.6259x over previous
"""Trainium2 Bass kernel for nn_Attention_66563403153646.

Dense transformer attention block with rotary embeddings + gated adapter
(prefix) attention, fp32 reference:

    y = softmax(rope(x@wq) @ rope(x@wk).T * k + mask) @ (x@wv)
      + gate * softmax(rope(x@wq) @ (adapter@wk).T * k) @ (adapter@wv)
    out = y @ wo

Sharding: 4-way tensor-parallel over heads x 2-way data-parallel over batch
(8 NeuronCores). Each core computes a [S, D] partial of its batch's output
(its 8 heads' contribution through wo); the host sums the 4 TP partials.

Matmuls run as float32r (full fp32 data, PE reduced-precision fast mode;
1 cycle/row for moving dim >= 256 vs 4 for plain fp32). Layouts:
  - x is fed transposed ([D, S]) so projections contract D on partitions.
  - q/k are computed per-head in [HD, S] layout with rope-pair-permuted
    head dims (host permutes wq/wk columns: even rope dims first, odd
    second) so RoPE is two partition-halves of elementwise ops.
  - scores are computed per 128-row q block over 512-col k tiles
    (causal: only tiles <= diagonal; diagonal tile gets the mask band).
  - p blocks are PE-transposed so p@v contracts k on partitions,
    producing out in [HD, S] layout, which is exactly the lhsT layout
    the final wo matmul needs.
"""

import sys

sys.path.insert(0, "/opt/trn_rl_repo")

import math
from dataclasses import dataclass

import numpy as np

import concourse.bass as bass
import concourse.mybir as mybir
import concourse.tile as tile
from concourse import bacc
from concourse.masks import make_identity

f32 = mybir.dt.float32
f32r = mybir.dt.float32r

P = 128


@dataclass(frozen=True)
class Cfg:
    S: int = 2048  # sequence length
    D: int = 4096  # model dim
    HPC: int = 8  # heads per core
    HD: int = 128  # head dim
    AL: int = 10  # adapter len
    mm_dt: object = f32r  # matmul operand dtype

    @property
    def DC(self):  # D chunks of 128 (contraction)
        return self.D // P

    @property
    def DH(self):  # head-slice width
        return self.HPC * self.HD

    @property
    def NQ(self):  # 128-row q blocks
        return self.S // P

    @property
    def NT(self):  # 512-col tiles
        return self.S // 512


def build_nc(cfg: Cfg, phases=(1, 2, 3)):
    nc = bacc.Bacc(None, target_bir_lowering=False, debug=False)
    S, D, HPC, HD, AL = cfg.S, cfg.D, cfg.HPC, cfg.HD, cfg.AL
    DC, DH, NQ, NT = cfg.DC, cfg.DH, cfg.NQ, cfg.NT
    mdt = cfg.mm_dt
    inv_sqrt = 1.0 / math.sqrt(HD)

    # ---- I/O ----
    xT_d = nc.dram_tensor("xT", [D, S], mdt, kind="ExternalInput")
    wq_d = nc.dram_tensor("wq", [D, DH], mdt, kind="ExternalInput")
    wk_d = nc.dram_tensor("wk", [D, DH], mdt, kind="ExternalInput")
    wv_d = nc.dram_tensor("wv", [D, DH], mdt, kind="ExternalInput")
    wo_d = nc.dram_tensor("wo", [DH, D], mdt, kind="ExternalInput")
    adT_d = nc.dram_tensor("adT", [D, AL], mdt, kind="ExternalInput")
    cosT_d = nc.dram_tensor("cosT", [HD // 2, S], f32, kind="ExternalInput")
    sinT_d = nc.dram_tensor("sinT", [HD // 2, S], f32, kind="ExternalInput")
    # transposed 0/1 exp-mask band: for k block j, q rows of tile j//4
    mb_d = nc.dram_tensor("emaskT", [NQ, P, 512], f32, kind="ExternalInput")
    gate_d = nc.dram_tensor("gateb", [HPC, P, 1], f32, kind="ExternalInput")
    y_d = nc.dram_tensor("y", [S, D], f32, kind="ExternalOutput")

    ExpF = mybir.ActivationFunctionType.Exp
    AX = mybir.AxisListType.X
    Mul = mybir.AluOpType.mult
    Add = mybir.AluOpType.add

    with tile.TileContext(nc) as tc:
        with (
            tc.tile_pool(name="persist", bufs=1) as persist,
            tc.tile_pool(name="dram", bufs=1, space="DRAM") as dram,
        ):
            # persistent small tiles (cos on partitions 0:64, sin on 64:128)
            cs_sb = persist.tile([P, S], f32)
            HH = HD // 2
            nc.sync.dma_start(cs_sb[0:HH, :], cosT_d[:])
            nc.sync.dma_start(cs_sb[HH:, :], sinT_d[:])
            g_all = persist.tile([P, HPC], f32)
            for h in range(HPC):
                nc.sync.dma_start(g_all[:, h : h + 1], gate_d[h])
            adT_sb = persist.tile([P, DC, AL], mdt)
            nc.sync.dma_start(
                adT_sb[:], adT_d[:].rearrange("(c p) a -> p c a", p=P)
            )
            ident = persist.tile([P, P], f32)
            make_identity(nc, ident)
            ones_f = persist.tile([P, 1], f32)
            nc.vector.memset(ones_f[:], 1.0)
            ones_c = persist.tile([P, 1], mdt)
            nc.vector.tensor_copy(ones_c[:], ones_f[:])
            ones_r1 = persist.tile([1, P], f32)
            nc.vector.memset(ones_r1[:], 1.0)

            # DRAM scratch
            qT_dr = dram.tile([HPC, P, S], mdt)
            kT_dr = dram.tile([HPC, P, S], mdt)
            vT_dr = dram.tile([HPC, P, S], f32)
            oT_dr = dram.tile([HPC, P, S], mdt)
            akT_dr = dram.tile([HPC, P, AL], mdt)
            av_dr = dram.tile([HPC, AL, P], mdt)

            # ================= Phase 1: projections + rope =================
            XS = min(1024, S)  # x s-tile width
            NXS = S // XS if 1 in phases else 0
            with (
                tc.tile_pool(name="p1x", bufs=1) as p1x,
                tc.tile_pool(name="p1w", bufs=2) as p1w,
                tc.tile_pool(name="p1o", bufs=2) as p1o,
                tc.tile_pool(name="p1t", bufs=1) as p1t,
                tc.tile_pool(name="p1ps", bufs=4, space="PSUM") as p1ps,
                tc.tile_pool(name="p1psa", bufs=1, space="PSUM") as p1psa,
            ):
                for st in range(NXS):
                    xt = p1x.tile([P, DC, XS], mdt, tag="xt")
                    nc.sync.dma_start(
                        xt[:],
                        xT_d[:, st * XS : (st + 1) * XS].rearrange(
                            "(c p) s -> p c s", p=P
                        ),
                    )
                    for h in range(HPC):
                        for proj, w_dram, out_dr in (
                            ("q", wq_d, qT_dr),
                            ("k", wk_d, kT_dr),
                            ("v", wv_d, vT_dr),
                        ):
                            wt = p1w.tile([P, DC, P], mdt, tag="wt")
                            nc.sync.dma_start(
                                wt[:],
                                w_dram[:, h * HD : (h + 1) * HD].rearrange(
                                    "(c p) m -> p c m", p=P
                                ),
                            )
                            # adapter projections (once, while w resident)
                            if st == 0 and proj == "k":
                                ps_ak = p1psa.tile([P, AL], f32, tag="ps_ak")
                                for c in range(DC):
                                    nc.tensor.matmul(
                                        ps_ak[:],
                                        wt[:, c, :],
                                        adT_sb[:, c, :],
                                        start=(c == 0),
                                        stop=(c == DC - 1),
                                    )
                                akT_sb = p1o.tile([P, AL], mdt, tag="akT")
                                nc.vector.tensor_copy(akT_sb[:], ps_ak[:])
                                nc.gpsimd.dma_start(akT_dr[h], akT_sb[:])
                            if st == 0 and proj == "v":
                                ps_av = p1psa.tile([AL, P], f32, tag="ps_av")
                                for c in range(DC):
                                    nc.tensor.matmul(
                                        ps_av[:],
                                        adT_sb[:, c, :],
                                        wt[:, c, :],
                                        start=(c == 0),
                                        stop=(c == DC - 1),
                                    )
                                av_sb = p1o.tile([AL, P], mdt, tag="av")
                                nc.vector.tensor_copy(av_sb[:], ps_av[:])
                                nc.gpsimd.dma_start(av_dr[h], av_sb[:])

                            for sh in range(XS // 512):
                                soff = st * XS + sh * 512
                                psum = p1ps.tile([P, 512], f32, tag="p1psum")
                                for c in range(DC):
                                    nc.tensor.matmul(
                                        psum[:],
                                        wt[:, c, :],
                                        xt[:, c, sh * 512 : (sh + 1) * 512],
                                        start=(c == 0),
                                        stop=(c == DC - 1),
                                    )
                                if proj == "v":
                                    vt_sb = p1o.tile([P, 512], f32, tag="vt")
                                    nc.scalar.copy(vt_sb[:], psum[:])
                                    nc.gpsimd.dma_start(
                                        out_dr[h, :, soff : soff + 512], vt_sb[:]
                                    )
                                else:
                                    # rope: psum partitions 0:64 = even dims
                                    # (x0), 64:128 = odd dims (x1). All four
                                    # products go to base-0 tmp tiles (PSUM
                                    # x SBUF inputs may differ in base
                                    # partition; SBUF x SBUF may not), the
                                    # two combines are then base-aligned.
                                    c_ap = cs_sb[0:HH, soff : soff + 512]
                                    s_ap = cs_sb[HH:, soff : soff + 512]
                                    x0 = psum[0:HH, :]
                                    x1 = psum[HH : 2 * HH, :]
                                    ta = p1t.tile([HH, 512], f32, tag="ta")
                                    tb = p1t.tile([HH, 512], f32, tag="tb")
                                    tc2 = p1t.tile([HH, 512], f32, tag="tc")
                                    td = p1t.tile([HH, 512], f32, tag="td")
                                    qt_sb = p1o.tile([P, 512], mdt, tag="qkt")
                                    nc.vector.tensor_tensor(ta[:], x0, c_ap, op=Mul)
                                    nc.vector.tensor_tensor(tb[:], x1, s_ap, op=Mul)
                                    nc.vector.tensor_sub(qt_sb[0:HH, :], ta[:], tb[:])
                                    nc.vector.tensor_tensor(tc2[:], x0, s_ap, op=Mul)
                                    nc.vector.tensor_tensor(td[:], x1, c_ap, op=Mul)
                                    nc.vector.tensor_add(qt_sb[HH:, :], tc2[:], td[:])
                                    nc.gpsimd.dma_start(
                                        out_dr[h, :, soff : soff + 512], qt_sb[:]
                                    )

            # ================= Phase 2: attention per head =================
            # scoresT layout [k, q]: p = exp(kT_blk.T @ qT_tile * inv_sqrt)
            # lands directly in the layout p@v needs -- no p transposes.
            # Scores are O(5) so exp needs no max subtraction; causal masking
            # multiplies the diagonal-band blocks by a 0/1 mask; per-q sums
            # come from a ones-row matmul and normalization happens at
            # eviction via a K=1 broadcast matmul of 1/sums.
            HPC2 = HPC if 2 in phases else 0
            with (
                tc.tile_pool(name="p2qkv", bufs=2) as p2qkv,
                tc.tile_pool(name="p2vn", bufs=2) as p2vn,
                tc.tile_pool(name="p2pt", bufs=2) as p2pt,
                tc.tile_pool(name="p2sm", bufs=4) as p2sm,
                tc.tile_pool(name="p2o", bufs=2) as p2o,
                tc.tile_pool(name="p2mask", bufs=1) as p2mask,
                tc.tile_pool(name="p2ps_s", bufs=2, space="PSUM") as p2ps_s,
                tc.tile_pool(name="p2ps_o", bufs=2, space="PSUM") as p2ps_o,
                tc.tile_pool(name="p2ps_t", bufs=1, space="PSUM") as p2ps_t,
                tc.tile_pool(name="p2ps_su", bufs=1, space="PSUM") as p2ps_su,
                tc.tile_pool(name="p2ps_b", bufs=2, space="PSUM") as p2ps_b,
            ):
                emask_sb = p2mask.tile([P, NQ, 512], f32)
                nc.sync.dma_start(
                    emask_sb[:], mb_d[:].rearrange("i p m -> p i m")
                )

                def emit_pv(ph, pQ, ptb, apT, v_nat, av):
                    """sums + normalize-broadcast + p@v + adapter + evict."""
                    nkb = (pQ + 1) * 4
                    ps_su = p2ps_su.tile([1, 512], f32, tag="ps_su")
                    for jb in range(nkb):
                        nc.tensor.matmul(
                            ps_su[:],
                            ones_c[:],
                            ptb[:, jb, :],
                            start=(jb == 0),
                            stop=(jb == nkb - 1),
                        )
                    rrow = p2sm.tile([1, 512], f32, tag="rrow")
                    nc.vector.reciprocal(rrow[:], ps_su[:])
                    ps_bc = p2ps_b.tile([P, 512], f32, tag="ps_b")
                    nc.tensor.matmul(
                        ps_bc[:], ones_r1[:], rrow[:], start=True, stop=True
                    )
                    ps_o = p2ps_o.tile([P, 512], f32, tag="ps_o")
                    for jb in range(nkb):
                        nc.tensor.matmul(
                            ps_o[:],
                            v_nat[:, jb, :],
                            ptb[:, jb, :],
                            start=(jb == 0),
                            stop=(jb == nkb - 1),
                        )
                    ps_a2 = p2ps_b.tile([P, 512], f32, tag="ps_b")
                    nc.tensor.matmul(ps_a2[:], av[:], apT[:], start=True, stop=True)
                    bc_sb = p2o.tile([P, 512], f32, tag="bc_sb")
                    nc.any.tensor_copy(bc_sb[:], ps_bc[:])
                    oT_sb = p2o.tile([P, 512], mdt, tag="oT")
                    nc.vector.scalar_tensor_tensor(
                        oT_sb[:], ps_o[:], 1.0, bc_sb[:], op0=Mul, op1=Mul
                    )
                    nc.vector.tensor_add(oT_sb[:], oT_sb[:], ps_a2[:])
                    nc.gpsimd.dma_start(
                        oT_dr[ph, :, pQ * 512 : (pQ + 1) * 512], oT_sb[:]
                    )

                pending = None
                for h in range(HPC2):
                    qT = p2qkv.tile([P, S], mdt, tag="qT")
                    kT = p2qkv.tile([P, S], mdt, tag="kT")
                    vT = p2qkv.tile([P, S], f32, tag="vT")
                    nc.sync.dma_start(qT[:], qT_dr[h])
                    nc.sync.dma_start(kT[:], kT_dr[h])
                    nc.sync.dma_start(vT[:], vT_dr[h])
                    akT = p2sm.tile([P, AL], mdt, tag="akT2")
                    av = p2sm.tile([AL, P], mdt, tag="av2")
                    nc.sync.dma_start(akT[:], akT_dr[h])
                    nc.sync.dma_start(av[:], av_dr[h])

                    # v natural layout [s-block, NQ, d]
                    v_nat = p2vn.tile([P, NQ, P], mdt, tag="v_nat")
                    for sb_i in range(NQ):
                        ps_vt = p2ps_t.tile([P, P], f32, tag="ps_t")
                        nc.tensor.transpose(
                            ps_vt[:], vT[:, sb_i * P : (sb_i + 1) * P], ident[:]
                        )
                        nc.any.tensor_copy(v_nat[:, sb_i, :], ps_vt[:])

                    for Q in range(NT):
                        nkb = (Q + 1) * 4
                        qtile = qT[:, Q * 512 : (Q + 1) * 512]
                        ptb = p2pt.tile([P, NQ, 512], mdt, tag="ptb")
                        apT = p2sm.tile([AL, 512], mdt, tag="apT")
                        for jb in range(nkb):
                            ps_s = p2ps_s.tile([P, 512], f32, tag="ps_s")
                            nc.tensor.matmul(
                                ps_s[:],
                                kT[:, jb * P : (jb + 1) * P],
                                qtile,
                                start=True,
                                stop=True,
                            )
                            nc.scalar.activation(
                                ptb[:, jb, :],
                                ps_s[:],
                                ExpF,
                                bias=0.0,
                                scale=inv_sqrt,
                            )
                            if jb // 4 == Q:
                                nc.vector.tensor_mul(
                                    ptb[:, jb, :],
                                    ptb[:, jb, :],
                                    emask_sb[:, jb, :],
                                )
                        # adapter (natural layout, per q block)
                        for qb in range(4):
                            i = Q * 4 + qb
                            ps_a = p2ps_t.tile([P, AL], f32, tag="ps_t")
                            nc.tensor.matmul(
                                ps_a[:],
                                qT[:, i * P : (i + 1) * P],
                                akT[:],
                                start=True,
                                stop=True,
                            )
                            asm = p2sm.tile([P, AL], f32, tag="asm")
                            asum = p2sm.tile([P, 1], f32, tag="asum")
                            nc.scalar.activation(
                                asm[:],
                                ps_a[:],
                                ExpF,
                                bias=0.0,
                                scale=inv_sqrt,
                                accum_out=asum[:],
                            )
                            arec = p2sm.tile([P, 1], f32, tag="arec")
                            nc.vector.reciprocal(arec[:], asum[:])
                            nc.vector.tensor_tensor(
                                arec[:], arec[:], g_all[:, h : h + 1], op=Mul
                            )
                            nc.any.tensor_scalar_mul(asm[:], asm[:], arec[:])
                            ps_apt = p2ps_t.tile([P, P], f32, tag="ps_t")
                            nc.tensor.transpose(ps_apt[:AL, :], asm[:], ident[:])
                            nc.any.tensor_copy(
                                apT[:, qb * P : (qb + 1) * P], ps_apt[:AL, :]
                            )

                        # pipeline: heavy tail of the PREVIOUS q-tile now
                        if pending is not None:
                            emit_pv(*pending)
                        pending = (h, Q, ptb, apT, v_nat, av)
                if pending is not None:
                    emit_pv(*pending)

            # ================= Phase 3: out @ wo =================
            HPC3 = HPC if 3 in phases else 0
            with (
                tc.tile_pool(name="p3o", bufs=1) as p3o,
                tc.tile_pool(name="p3w", bufs=2) as p3w,
                tc.tile_pool(name="p3y", bufs=3) as p3y,
                tc.tile_pool(name="p3ps", bufs=4, space="PSUM") as p3ps,
            ):
                oTs = []
                for h in range(HPC3):
                    ot = p3o.tile([P, S], mdt, name=f"p3ot{h}")
                    nc.sync.dma_start(ot[:], oT_dr[h])
                    oTs.append(ot)
                WC = DH // P  # wo row chunks == HPC
                for et in range(D // 512 if 3 in phases else 0):
                    wo_t = p3w.tile([P, WC, 512], mdt, tag="wo_t")
                    nc.sync.dma_start(
                        wo_t[:],
                        wo_d[:, et * 512 : (et + 1) * 512].rearrange(
                            "(c p) e -> p c e", p=P
                        ),
                    )
                    for st in range(NQ):
                        ps_y = p3ps.tile([P, 512], f32, tag="ps_y")
                        for h in range(HPC):
                            nc.tensor.matmul(
                                ps_y[:],
                                oTs[h][:, st * P : (st + 1) * P],
                                wo_t[:, h, :],
                                start=(h == 0),
                                stop=(h == HPC - 1),
                            )
                        y_sb = p3y.tile([P, 512], f32, tag="y_sb")
                        nc.scalar.copy(y_sb[:], ps_y[:])
                        nc.gpsimd.dma_start(
                            y_d[st * P : (st + 1) * P, et * 512 : (et + 1) * 512],
                            y_sb[:],
                        )

    nc.compile()
    return nc


# ====================== host side: sharding + runner ======================

B, S, D, H = 2, 2048, 4096, 32
HD = D // H
AL = 10
N_CORES = 8
TP = 4  # head groups
HPC = H // TP  # 8 heads per core

_RUNNER = None


def _make_runner(nc, n_cores=N_CORES):
    import jax
    from jax.sharding import Mesh, PartitionSpec
    from jax.experimental.shard_map import shard_map

    from concourse import bass2jax
    from concourse.bass2jax import _bass_exec_p, install_neuronx_cc_hook

    install_neuronx_cc_hook()
    partition_name = nc.partition_id_tensor.name if nc.partition_id_tensor else None

    in_names, out_names, out_avals = [], [], []
    for alloc in nc.m.functions[0].allocations:
        if not isinstance(alloc, mybir.MemoryLocationSet):
            continue
        name = alloc.memorylocations[0].name
        if alloc.kind == "ExternalInput":
            if name != partition_name:
                in_names.append(name)
        elif alloc.kind == "ExternalOutput":
            out_names.append(name)
            out_avals.append(
                jax.core.ShapedArray(
                    tuple(alloc.tensor_shape), mybir.dt.np(alloc.dtype)
                )
            )
    n_params = len(in_names)
    n_outs = len(out_avals)
    all_in_names = list(in_names) + list(out_names)
    if partition_name is not None:
        all_in_names.append(partition_name)

    def _body(*args):
        operands = list(args)
        if partition_name is not None:
            operands.append(bass2jax.partition_id_tensor())
        outs = _bass_exec_p.bind(
            *operands,
            out_avals=tuple(out_avals),
            in_names=tuple(all_in_names),
            out_names=tuple(out_names),
            lowering_input_output_aliases=(),
            sim_require_finite=True,
            sim_require_nnan=True,
            nc=nc,
        )
        return tuple(outs)

    devices = jax.devices()[:n_cores]
    mesh = Mesh(np.asarray(devices), ("core",))
    fn = jax.jit(
        shard_map(
            _body,
            mesh=mesh,
            in_specs=(PartitionSpec("core"),) * (n_params + n_outs),
            out_specs=(PartitionSpec("core"),) * n_outs,
            check_rep=False,
        ),
        keep_unused=True,
    )

    class Runner:
        in_names_ = in_names
        out_names_ = out_names

        def prep(self, in_maps):
            import jax as _jax

            concat_in = [
                np.concatenate(
                    [np.ascontiguousarray(in_maps[c][n]) for c in range(n_cores)],
                    axis=0,
                )
                for n in in_names
            ]
            concat_zero = [
                np.zeros((n_cores * a.shape[0], *a.shape[1:]), a.dtype)
                for a in out_avals
            ]
            shardings = [
                _jax.sharding.NamedSharding(mesh, PartitionSpec("core"))
            ] * (n_params + n_outs)
            return _jax.device_put(concat_in + concat_zero, shardings)

        def run(self, args):
            import jax as _jax

            outs = fn(*args)
            _jax.block_until_ready(outs)
            return [
                {
                    n: np.asarray(outs[i]).reshape(n_cores, *out_avals[i].shape)[c]
                    for i, n in enumerate(out_names)
                }
                for c in range(n_cores)
            ]

        def time_pipelined(self, args, reps=10, warmup=1):
            import time as _time

            import jax as _jax

            for _ in range(warmup):
                _jax.block_until_ready(fn(*args))
            t0 = _time.perf_counter()
            outs = None
            for _ in range(reps):
                outs = fn(*args)
            _jax.block_until_ready(outs)
            return (_time.perf_counter() - t0) / reps

    return Runner()


def _shard_inputs(x, cos, sin, mask, wq, wk, wv, wo, gate, adapter):
    """Build the 8 per-core input maps."""
    # rope permutation of head-dim columns: even dims first, odd second
    perm = np.concatenate(
        [np.arange(0, HD, 2), np.arange(1, HD, 2)]
    )  # within one head
    col_perm = np.concatenate(
        [h * HD + perm for h in range(H)]
    )  # all heads, head-major
    wq_p = np.ascontiguousarray(wq[:, col_perm], dtype=np.float32)
    wk_p = np.ascontiguousarray(wk[:, col_perm], dtype=np.float32)

    cosT = np.ascontiguousarray(cos.T, dtype=np.float32)  # [64, S]
    sinT = np.ascontiguousarray(sin.T, dtype=np.float32)
    adT = np.ascontiguousarray(adapter[0].T, dtype=np.float32)  # [D, AL]

    m = np.asarray(mask, dtype=np.float32)[0, 0]  # [S, S]
    NQ = S // P
    # emaskT[j] = 0/1 allowed-mask of k block j vs its diagonal q tile,
    # transposed to [k, q]
    emaskT = np.empty((NQ, P, 512), dtype=np.float32)
    for j in range(NQ):
        Qt = j // 4
        emaskT[j] = (
            m[Qt * 512 : (Qt + 1) * 512, j * P : (j + 1) * P].T == 0
        ).astype(np.float32)

    gate_v = np.asarray(gate, dtype=np.float32).reshape(H)  # per head

    xT = [
        np.ascontiguousarray(np.asarray(x[b], dtype=np.float32).T) for b in range(B)
    ]

    in_maps = []
    for c in range(N_CORES):
        b = c // TP
        g = c % TP
        hs = g * HPC * HD  # column slice start
        gateb = np.ascontiguousarray(
            np.repeat(gate_v[g * HPC : (g + 1) * HPC, None], P, axis=1)[..., None]
        ).astype(np.float32)  # [HPC, P, 1]
        in_maps.append(
            {
                "xT": xT[b],
                "wq": np.ascontiguousarray(wq_p[:, hs : hs + HPC * HD]),
                "wk": np.ascontiguousarray(wk_p[:, hs : hs + HPC * HD]),
                "wv": np.ascontiguousarray(
                    np.asarray(wv, dtype=np.float32)[:, hs : hs + HPC * HD]
                ),
                "wo": np.ascontiguousarray(
                    np.asarray(wo, dtype=np.float32)[hs : hs + HPC * HD, :]
                ),
                "adT": adT,
                "cosT": cosT,
                "sinT": sinT,
                "emaskT": emaskT,
                "gateb": gateb,
            }
        )
    return in_maps


def get_runner():
    global _RUNNER
    if _RUNNER is None:
        nc = build_nc(Cfg())
        _RUNNER = _make_runner(nc)
    return _RUNNER


def kernel(**inputs) -> np.ndarray:
    x = np.asarray(inputs["x"])
    in_maps = _shard_inputs(
        x,
        inputs["cos"],
        inputs["sin"],
        inputs["mask"],
        inputs["wq"],
        inputs["wk"],
        inputs["wv"],
        inputs["wo"],
        inputs["gate"],
        inputs["adapter"],
    )
    runner = get_runner()
    args = runner.prep(in_maps)
    outs = runner.run(args)
    y = np.zeros((B, S, D), dtype=np.float32)
    for c in range(N_CORES):
        y[c // TP] += outs[c]["y"]
    return y
